# revision 37
# baseline (speedup 1.0000x reference)
"""MultiHeadAttention Trainium2 Bass kernel (v3: fp16 interleaved pipeline).

Problem: B=8, H=W=32 (S=1024), C=512, 8 heads x 64 dim.
Sharding: data-parallel over batch, one batch element per NeuronCore (8 cores).

Design:
- All matmul operands fp16 (10 mantissa bits), fp32 accumulation in PSUM.
  x and W are DMA'd as fp32 and converted once (x on DVE, W lazily per
  128-col chunk on DVE just before first use).
- Input transposes on the PE in fp16 (1 cycle/col, FWL-eligible weight
  loads that hide under the stream).
- Projections: Q/K use fp16 W-chunk stationary with fp16 xT moving
  (transposed outputs QT/KT [d, s]); V uses fp16 xT stationary with fp16 W
  moving (natural [s, d]), with a rank-1 ones x bias matmul folded into the
  accumulation group and the softmax-denominator ones column baked into
  V_aug (padded to 128 stationary cols for fast weight load).
- Attention per head pair: scoresT [kpos, q] matmuls (K=64, head pair at
  PE row bases 0/64), exp on ACT from PSUM in [128, 2, 512] groups with
  the 1/8 scale folded in (scores ~N(0,1), no max subtraction), bf16 attT
  (fp16 ACT writes are ~30% slower), V_aug-stationary attV accumulation,
  PE back-transpose + DVE reciprocal/scale into a staged fp32 output.
- Single-phase emission: x DMA chunks, converts, transposes and projection
  d-chunks are interleaved into the attention stream as filler units
  (emitted after the attv pair so a dependency-stalled filler never blocks
  ready work); attv lags scores by two kt groups so the PE never waits on
  the exp it just fed; per-sub-block tails (attv kt=7 + evacuation) are
  deferred into the next sub-block; output DMA'd eagerly per head pair.
- DMA priority order feeds the serial critical path: xk, Wk, Wq, xq(lo),
  biases, Wv, xv, xq(hi).

Measured: ~148 us on trn2 (baseline v1: ~150 us), absmax rel err 1.6e-3
vs fp32 reference (threshold 2e-2; numpy-simulated precision matches).
"""
import sys

import numpy as np

if "/opt/trn_rl_repo" not in sys.path:
    sys.path.insert(0, "/opt/trn_rl_repo")

import concourse.bacc as bacc
import concourse.mybir as mybir
import concourse.tile as tile
from concourse import masks
from concourse.bass_utils import run_bass_kernel_spmd

B, HS, WS, C = 8, 32, 32, 512
S = HS * WS          # 1024
D = 512
HEADS = 8
HD = 64              # head dim
N_CORES = 8

f32 = mybir.dt.float32
f16 = mybir.dt.float16
bf16 = mybir.dt.bfloat16
Exp = mybir.ActivationFunctionType.Exp


def build_nc():
    nc = bacc.Bacc("TRN2", target_bir_lowering=False, debug=False,
                   num_devices=N_CORES)

    x_d = {}
    w_d = {}
    b_d = {}
    for name in ("q", "k", "v"):
        x_d[name] = nc.dram_tensor(f"{name}_in", [S, C], f32, kind="ExternalInput")
        w_d[name] = nc.dram_tensor(f"W{name}", [C, D], f32, kind="ExternalInput")
        b_d[name] = nc.dram_tensor(f"b{name}", [D], f32, kind="ExternalInput")
    out_d = nc.dram_tensor("out", [S, D], f32, kind="ExternalOutput")

    with tile.TileContext(nc) as tc:
        with (
            tc.tile_pool(name="const", bufs=1) as cpool,
            tc.tile_pool(name="x32", bufs=5) as x32_pool,
            tc.tile_pool(name="xb", bufs=3) as xb_pool,
            tc.tile_pool(name="xT", bufs=1) as xt_pool,
            tc.tile_pool(name="wbuf", bufs=1) as w_pool,
            tc.tile_pool(name="proj", bufs=1) as proj_pool,
            tc.tile_pool(name="att", bufs=4) as att_pool,
            tc.tile_pool(name="ot", bufs=4) as ot_pool,
            tc.tile_pool(name="ps_p", bufs=2, space="PSUM") as ps_p,
            tc.tile_pool(name="ps_s", bufs=2, space="PSUM") as ps_s,
            tc.tile_pool(name="ps_o", bufs=2, space="PSUM") as ps_o,
        ):
            # ---------------- constants ----------------
            ident_f16 = cpool.tile([128, 128], f16)
            masks.make_identity(nc, ident_f16[:])
            ident_f32 = cpool.tile([128, 128], f32)
            masks.make_identity(nc, ident_f32[:])
            ones_sb = cpool.tile([128, 512], f32)
            nc.vector.memset(ones_sb[:], 1.0)
            ones_16 = cpool.tile([1, 512], f16)
            nc.vector.tensor_copy(ones_16[:], ones_sb[0:1, :])

            # ---------------- persistent tensors ----------------
            QT = proj_pool.tile([128, 4, S], f16, name="QT")  # [d%128, d//128, s]
            KT = proj_pool.tile([128, 4, S], f16, name="KT")
            # V_aug padded: [s%128, s//128, head, 128]; col 64 = 1.0 (denom),
            # cols 65.. = 0 so the 128-wide stationary gets FWL.
            V = proj_pool.tile([128, 8, HEADS, 128], f16, name="V")
            zz = cpool.tile([128, 512], f16)
            nc.vector.memset(zz[:], 0.0)
            for st8 in range(8):
                nc.vector.tensor_copy(
                    V[:, st8, :, HD + 1:],
                    zz[:, 0:8 * 63].rearrange("p (a o) -> p a o", a=8))
            nc.vector.tensor_copy(
                V[:, :, :, HD:HD + 1],
                ones_sb[:, 0:64].rearrange("p (a b o) -> p a b o", a=8, b=8))
            o_stage = proj_pool.tile([128, 8, D], f32, name="o_stage")

            # xT tiles (f16): [c%128, c//128, s]
            xT = {}
            for name in ("q", "k", "v"):
                xT[name] = xt_pool.tile([128, 4, S], f16, name=f"xT_{name}",
                                        tag=f"xT_{name}")

            # ---------------- input DMAs (priority order) -------------------
            # x chunks: lists of t-indices; k's first chunk is small so the
            # transpose chain starts as early as possible.
            CHUNKS = {"k": [(0, 1, 2, 3), (4, 5, 6, 7)],
                      "q": [(0, 1, 2, 3), (4, 5, 6, 7)],
                      "v": [(0, 1, 2, 3), (4, 5, 6, 7)]}
            chunk_of = {(n, t): ci for n, chs in CHUNKS.items()
                        for ci, ts in enumerate(chs) for t in ts}
            x32 = {}   # (name, ch) -> tile
            def dma_x(name, ch):
                ts = CHUNKS[name][ch]
                t_ = x32_pool.tile([128, len(ts), C], f32,
                                   name=f"x32_{name}{ch}", tag="x32",
                                   padded_shape=[128, 4, C])
                x_r = x_d[name][:].rearrange("(t p) c -> p t c", p=128)
                nc.sync.dma_start(t_[:], x_r[:, ts[0]:ts[0] + len(ts), :])
                x32[(name, ch)] = t_

            w_sb = {}
            w16 = {}
            def dma_w(name):
                w_sb[name] = w_pool.tile([128, 4, D], f32, name=f"w_{name}",
                                         tag=f"w_{name}")
                nc.sync.dma_start(
                    w_sb[name][:],
                    w_d[name][:].rearrange("(cc p) d -> p cc d", p=128))
                w16[name] = w_pool.tile([128, 4, D], f16, name=f"w16_{name}",
                                        tag=f"w16_{name}")

            # lazy per-chunk W fp16 converts on the scalar engine
            w16_done = set()

            def cvt_w(name, dt):
                if (name, dt) in w16_done:
                    return
                w16_done.add((name, dt))
                nc.vector.tensor_copy(
                    w16[name][:, :, dt * 128:(dt + 1) * 128],
                    w_sb[name][:, :, dt * 128:(dt + 1) * 128])

            def cvt_wv_half(half):
                if ("v", half) in w16_done:
                    return
                w16_done.add(("v", half))
                nc.vector.tensor_copy(
                    w16["v"][:, :, half * 256:(half + 1) * 256],
                    w_sb["v"][:, :, half * 256:(half + 1) * 256])

            dma_x("k", 0)
            dma_x("k", 1)
            dma_w("k")
            dma_w("q")
            dma_x("q", 0)
            b_sb = {}
            for name in ("k", "q"):
                b_sb[name] = w_pool.tile([128, 4], f32, name=f"b_{name}",
                                         tag=f"b_{name}")
                nc.sync.dma_start(
                    b_sb[name][:], b_d[name][:].rearrange("(dt p) -> p dt", p=128))
            bv_sb = w_pool.tile([1, D], f32, name="bv_sb", tag="bv_sb")
            nc.sync.dma_start(
                bv_sb[:], b_d["v"][:].rearrange("(o d) -> o d", o=1))
            bv_16 = w_pool.tile([1, D], f16, name="bv_16", tag="bv_16")
            nc.scalar.copy(bv_16[:], bv_sb[:])
            dma_w("v")
            dma_x("v", 0)
            dma_x("v", 1)
            dma_x("q", 1)

            # ---------------- emission helpers ----------------
            xb = {}

            def cv(name, ch):
                """Convert x chunk to fp16 on DVE."""
                ts = CHUNKS[name][ch]
                t_ = xb_pool.tile([128, len(ts), C], f16,
                                  name=f"xb_{name}{ch}", tag="xb",
                                  padded_shape=[128, 4, C])
                nc.vector.tensor_copy(t_[:], x32[(name, ch)][:])
                xb[(name, ch)] = t_

            def T_t(name, t):
                """PE-transpose x rows t*128..t*128+128 into xT[name]."""
                ch = chunk_of[(name, t)]
                src = xb[(name, ch)]
                ti = t - CHUNKS[name][ch][0]
                pst = ps_p.tile([128, 4, 128], f16, tag="pp",
                                name=f"pst_{name}_{t}")
                for cc in range(4):
                    nc.tensor.transpose(
                        pst[:, cc, :],
                        src[:, ti, cc * 128:(cc + 1) * 128],
                        ident_f16[:])
                nc.vector.tensor_copy(
                    xT[name][:, :, t * 128:(t + 1) * 128], pst[:])

            def projqk(name, dt, qhs=(0, 1)):
                """Q/K projection d-chunk dt for the given q-halves.

                stationary: fp16 W slice, moving: fp16 xT.
                """
                tgt = QT if name == "q" else KT
                psqs = {}
                for qh in qhs:
                    psqs[qh] = ps_p.tile([128, 512], f32, tag="pp",
                                         name=f"psq_{name}{dt}_{qh}")
                for cc in range(4):
                    for qh in qhs:
                        nc.tensor.matmul(
                            psqs[qh][:],
                            w16[name][:, cc, dt * 128:(dt + 1) * 128],
                            xT[name][:, cc, qh * 512:(qh + 1) * 512],
                            start=(cc == 0), stop=(cc == 3))
                for qh in qhs:
                    nc.vector.tensor_scalar_add(
                        tgt[:, dt, qh * 512:(qh + 1) * 512], psqs[qh][:],
                        b_sb[name][:, dt:dt + 1])

            def projv(st):
                """V projection s-chunk st: stationary xT_v fp16 (FWL),
                moving W fp16; ones-column bias matmul; evac to V_aug."""
                psv = ps_p.tile([128, 512], f32, tag="pp", name=f"psv_{st}")
                for cc in range(4):
                    nc.tensor.matmul(
                        psv[:],
                        xT["v"][:, cc, st * 128:(st + 1) * 128],
                        w16["v"][:, cc, :],
                        start=(cc == 0), stop=False)
                nc.tensor.matmul(
                    psv[:], ones_16[0:1, 0:128], bv_16[0:1, :],
                    start=False, stop=True)
                nc.vector.tensor_copy(
                    V[:, st, :, 0:HD],
                    psv[:].rearrange("p (h e) -> p h e", h=HEADS))

            # ---------------- filler queue ----------------
            fillers = []

            def fill(n=1):
                for _ in range(n):
                    if fillers:
                        fillers.pop(0)()

            def drain_fillers():
                while fillers:
                    fillers.pop(0)()

            # ---------------- attention ----------------
            # Deferred tail (attV kt=7 + evacuation) per sub-block, flushed
            # inside the next sub-block after its first exp.
            pend = [None]

            def scores_exp(hp, qh, kt):
                heads = (2 * hp, 2 * hp + 1)
                pss = ps_s.tile([128, 2, 512], f32, tag="pss",
                                name=f"pss_{hp}_{qh}_{kt}")
                for i, h in enumerate(heads):
                    po = (h % 2) * HD
                    nc.tensor.matmul(
                        pss[:, i, :],
                        KT[po:po + HD, hp, kt * 128:(kt + 1) * 128],
                        QT[po:po + HD, hp, qh * 512:(qh + 1) * 512],
                        start=True, stop=True)
                attT = att_pool.tile([128, 2, 512], bf16, tag="attT",
                                     name=f"attT_{hp}_{qh}_{kt}")
                nc.scalar.activation(attT[:], pss[:], Exp, scale=0.125)
                return attT

            def attention(hp):
                heads = (2 * hp, 2 * hp + 1)
                for qh in range(2):
                    atts = []
                    atts.append(scores_exp(hp, qh, 0))
                    if pend[0] is not None:
                        pend[0]()
                        pend[0] = None
                    pso = {}
                    for h in heads:
                        pso[h] = ps_o.tile([128, 512], f32,
                                           name=f"pso{h}_{qh}", tag="pso")
                    # software pipeline: attv lags scores by three groups
                    # so the PE never waits on the exp stream.
                    for kt in range(1, 8):
                        atts.append(scores_exp(hp, qh, kt))
                        if kt >= 3:
                            for i, h in enumerate(heads):
                                nc.tensor.matmul(
                                    pso[h][:],
                                    V[:, kt - 3, h, :],
                                    atts[kt - 3][:, i, :],
                                    start=(kt == 3), stop=False)
                        fill(1)
                    for kt in (5, 6):
                        for i, h in enumerate(heads):
                            nc.tensor.matmul(
                                pso[h][:], V[:, kt, h, :], atts[kt][:, i, :],
                                start=False, stop=False)

                    att7 = atts[7]

                    def tail(heads=heads, pso=pso, att7=att7, qh=qh):
                        for i, h in enumerate(heads):
                            nc.tensor.matmul(
                                pso[h][:], V[:, 7, h, :], att7[:, i, :],
                                start=False, stop=True)
                        for h in heads:
                            oT = ot_pool.tile([HD + 1, 512], f16, tag="oT",
                                              name=f"oT{h}_{qh}")
                            nc.vector.tensor_copy(oT[:], pso[h][0:HD + 1, :])
                            pbt = ps_o.tile([128, 4, HD + 2], f16, tag="pso",
                                            name=f"pbt{h}_{qh}")
                            for qs in range(4):
                                nc.tensor.transpose(
                                    pbt[:, qs, 0:HD + 1],
                                    oT[:, qs * 128:(qs + 1) * 128],
                                    ident_f16[0:HD + 1, 0:HD + 1])
                            rec = ot_pool.tile([128, 4], f32, tag="rec",
                                               name=f"rec{h}_{qh}")
                            nc.vector.reciprocal(rec[:], pbt[:, :, HD])
                            for qs in range(4):
                                qt = qh * 4 + qs
                                nc.vector.tensor_scalar_mul(
                                    o_stage[:, qt, h * HD:(h + 1) * HD],
                                    pbt[:, qs, 0:HD],
                                    rec[:, qs:qs + 1])

                    pend[0] = tail

            # ---------------- prefix ----------------
            cv("k", 0)
            for t in range(4):
                T_t("k", t)
            cv("k", 1)
            cvt_w("k", 0)
            for t in range(4, 8):
                T_t("k", t)
            projqk("k", 0)
            cv("q", 0)
            cvt_w("q", 0)
            for t in range(4):
                T_t("q", t)
            projqk("q", 0, qhs=(0,))
            cvt_wv_half(0)
            cvt_wv_half(1)
            cv("v", 0)
            T_t("v", 0)
            projv(0)
            T_t("v", 1)
            projv(1)

            # ---------------- filler schedule ----------------
            def u(*fns):
                def unit():
                    for f in fns:
                        f()
                return unit

            fillers.extend([
                # during (0, qh0): keep the V st chain >= 1 ahead of attv,
                # finish xT_q and QT dt0 qh1 (needed at (0, qh1) kt0).
                u(lambda: T_t("v", 2), lambda: projv(2)),
                u(lambda: T_t("v", 3), lambda: projv(3)),
                u(lambda: cv("v", 1), lambda: T_t("v", 4), lambda: projv(4),
                  lambda: T_t("v", 5), lambda: projv(5)),
                u(lambda: T_t("v", 6), lambda: projv(6)),
                u(lambda: T_t("v", 7), lambda: projv(7)),
                u(lambda: cv("q", 1)),
                u(lambda: [T_t("q", t) for t in range(4, 8)],
                  lambda: projqk("q", 0, qhs=(1,))),
                # during (0, qh1)
                u(lambda: cvt_w("k", 1), lambda: projqk("k", 1)),
                u(lambda: cvt_w("q", 1), lambda: projqk("q", 1)),
                u(lambda: cvt_w("k", 2), lambda: projqk("k", 2)),
                u(lambda: cvt_w("q", 2), lambda: projqk("q", 2)),
                u(lambda: cvt_w("k", 3), lambda: projqk("k", 3)),
                # during (1, qh0)
                u(lambda: cvt_w("q", 3), lambda: projqk("q", 3)),
            ])

            out_r = out_d[:].rearrange("(t p) d -> p t d", p=128)

            attention(0)
            attention(1)
            # hp0 tails flushed inside attention(1)'s first sub-block
            nc.sync.dma_start(out_r[:, :, 0:128], o_stage[:, :, 0:128])
            attention(2)
            nc.sync.dma_start(out_r[:, :, 128:256], o_stage[:, :, 128:256])
            attention(3)
            nc.sync.dma_start(out_r[:, :, 256:384], o_stage[:, :, 256:384])
            # hp3 qh0 rows were evacuated by the tail flushed inside (3, qh1)
            nc.sync.dma_start(out_r[:, 0:4, 384:512], o_stage[:, 0:4, 384:512])
            pend[0]()
            pend[0] = None
            drain_fillers()
            nc.sync.dma_start(out_r[:, 4:8, 384:512], o_stage[:, 4:8, 384:512])

    nc.compile()
    return nc


_NC = None


def _get_nc():
    global _NC
    if _NC is None:
        _NC = build_nc()
    return _NC


def _make_in_maps(inputs):
    in_maps = []
    for b in range(B):
        m = {
            "q_in": np.ascontiguousarray(np.asarray(inputs["q_in"])[b].reshape(S, C)),
            "k_in": np.ascontiguousarray(np.asarray(inputs["k_in"])[b].reshape(S, C)),
            "v_in": np.ascontiguousarray(np.asarray(inputs["v_in"])[b].reshape(S, C)),
            "Wq": np.asarray(inputs["Wq"]), "bq": np.asarray(inputs["bq"]),
            "Wk": np.asarray(inputs["Wk"]), "bk": np.asarray(inputs["bk"]),
            "Wv": np.asarray(inputs["Wv"]), "bv": np.asarray(inputs["bv"]),
        }
        in_maps.append(m)
    return in_maps


def kernel(**inputs):
    nc = _get_nc()
    res = run_bass_kernel_spmd(nc, _make_in_maps(inputs), list(range(N_CORES)))
    out = np.stack([res.results[i]["out"] for i in range(B)])
    return out.reshape(B, HS, WS, D).astype(np.float32)


if __name__ == "__main__":
    rng = np.random.default_rng(0)
    ins = {
        "q_in": rng.standard_normal((B, HS, WS, C), dtype=np.float32),
        "k_in": rng.standard_normal((B, HS, WS, C), dtype=np.float32),
        "v_in": rng.standard_normal((B, HS, WS, C), dtype=np.float32),
        "Wq": (rng.standard_normal((C, D)) / np.sqrt(C)).astype(np.float32),
        "Wk": (rng.standard_normal((C, D)) / np.sqrt(C)).astype(np.float32),
        "Wv": (rng.standard_normal((C, D)) / np.sqrt(C)).astype(np.float32),
        "bq": np.zeros(D, np.float32),
        "bk": np.zeros(D, np.float32),
        "bv": np.zeros(D, np.float32),
    }
    out = kernel(**ins)
    print("out shape:", out.shape, "finite:", np.isfinite(out).all())


# revision 38
# speedup vs baseline: 1.0059x; 1.0059x over previous
"""MultiHeadAttention Trainium2 Bass kernel (v3: fp16 interleaved pipeline).

Problem: B=8, H=W=32 (S=1024), C=512, 8 heads x 64 dim.
Sharding: data-parallel over batch, one batch element per NeuronCore (8 cores).

Design:
- All matmul operands fp16 (10 mantissa bits), fp32 accumulation in PSUM.
  x and W are DMA'd as fp32 and converted once (x on DVE, W lazily per
  128-col chunk on DVE just before first use).
- Input transposes on the PE in fp16 (1 cycle/col, FWL-eligible weight
  loads that hide under the stream).
- Projections: Q/K use fp16 W-chunk stationary with fp16 xT moving
  (transposed outputs QT/KT [d, s]); V uses fp16 xT stationary with fp16 W
  moving (natural [s, d]), with a rank-1 ones x bias matmul folded into the
  accumulation group and the softmax-denominator ones column baked into
  V_aug (padded to 128 stationary cols for fast weight load).
- Attention per head pair: scoresT [kpos, q] matmuls (K=64, head pair at
  PE row bases 0/64), exp on ACT from PSUM in [128, 2, 512] groups with
  the 1/8 scale folded in (scores ~N(0,1), no max subtraction), bf16 attT
  (fp16 ACT writes are ~30% slower), V_aug-stationary attV accumulation,
  PE back-transpose + DVE reciprocal/scale into a staged fp32 output.
- Single-phase emission: x DMA chunks, converts, transposes and projection
  d-chunks are interleaved into the attention stream as filler units
  (emitted after the attv pair so a dependency-stalled filler never blocks
  ready work); attv lags scores by two kt groups so the PE never waits on
  the exp it just fed; per-sub-block tails (attv kt=7 + evacuation) are
  deferred into the next sub-block; output DMA'd eagerly per head pair.
- DMA priority order feeds the serial critical path: xk, Wk, Wq, xq(lo),
  biases, Wv, xv, xq(hi).

Measured: ~146 us on trn2 (baseline v1: ~150 us), absmax rel err 2.1e-3
vs fp32 reference (threshold 2e-2; numpy-simulated precision matches).
"""
import sys

import numpy as np

if "/opt/trn_rl_repo" not in sys.path:
    sys.path.insert(0, "/opt/trn_rl_repo")

import concourse.bacc as bacc
import concourse.mybir as mybir
import concourse.tile as tile
from concourse import masks
from concourse.bass_utils import run_bass_kernel_spmd

B, HS, WS, C = 8, 32, 32, 512
S = HS * WS          # 1024
D = 512
HEADS = 8
HD = 64              # head dim
N_CORES = 8

f32 = mybir.dt.float32
f16 = mybir.dt.float16
bf16 = mybir.dt.bfloat16
Exp = mybir.ActivationFunctionType.Exp


def build_nc():
    nc = bacc.Bacc("TRN2", target_bir_lowering=False, debug=False,
                   num_devices=N_CORES)

    x_d = {}
    w_d = {}
    b_d = {}
    for name in ("q", "k", "v"):
        x_d[name] = nc.dram_tensor(f"{name}_in", [S, C], f32, kind="ExternalInput")
        w_d[name] = nc.dram_tensor(f"W{name}", [C, D], f32, kind="ExternalInput")
        b_d[name] = nc.dram_tensor(f"b{name}", [D], f32, kind="ExternalInput")
    out_d = nc.dram_tensor("out", [S, D], f32, kind="ExternalOutput")

    with tile.TileContext(nc) as tc:
        with (
            tc.tile_pool(name="const", bufs=1) as cpool,
            tc.tile_pool(name="x32", bufs=5) as x32_pool,
            tc.tile_pool(name="xb", bufs=3) as xb_pool,
            tc.tile_pool(name="xT", bufs=1) as xt_pool,
            tc.tile_pool(name="wbuf", bufs=1) as w_pool,
            tc.tile_pool(name="proj", bufs=1) as proj_pool,
            tc.tile_pool(name="att", bufs=4) as att_pool,
            tc.tile_pool(name="ot", bufs=4) as ot_pool,
            tc.tile_pool(name="ps_p", bufs=2, space="PSUM") as ps_p,
            tc.tile_pool(name="ps_s", bufs=2, space="PSUM") as ps_s,
            tc.tile_pool(name="ps_o", bufs=2, space="PSUM") as ps_o,
        ):
            # ---------------- constants ----------------
            ident_f16 = cpool.tile([128, 128], f16)
            masks.make_identity(nc, ident_f16[:])
            ident_f32 = cpool.tile([128, 128], f32)
            masks.make_identity(nc, ident_f32[:])
            ones_sb = cpool.tile([128, 512], f32)
            nc.vector.memset(ones_sb[:], 1.0)
            ones_16 = cpool.tile([1, 512], f16)
            nc.vector.tensor_copy(ones_16[:], ones_sb[0:1, :])

            # ---------------- persistent tensors ----------------
            QT = proj_pool.tile([128, 4, S], f16, name="QT")  # [d%128, d//128, s]
            KT = proj_pool.tile([128, 4, S], f16, name="KT")
            # V_aug padded: [s%128, s//128, head, 128]; col 64 = 1.0 (denom),
            # cols 65.. = 0 so the 128-wide stationary gets FWL.
            V = proj_pool.tile([128, 8, HEADS, 128], f16, name="V")
            zz = cpool.tile([128, 512], f16)
            nc.vector.memset(zz[:], 0.0)
            for st8 in range(8):
                nc.vector.tensor_copy(
                    V[:, st8, :, HD + 1:],
                    zz[:, 0:8 * 63].rearrange("p (a o) -> p a o", a=8))
            nc.vector.tensor_copy(
                V[:, :, :, HD:HD + 1],
                ones_sb[:, 0:64].rearrange("p (a b o) -> p a b o", a=8, b=8))
            o_stage = proj_pool.tile([128, 8, D], f32, name="o_stage")

            # xT tiles (f16): [c%128, c//128, s]
            xT = {}
            for name in ("q", "k", "v"):
                xT[name] = xt_pool.tile([128, 4, S], f16, name=f"xT_{name}",
                                        tag=f"xT_{name}")

            # ---------------- input DMAs (priority order) -------------------
            # x chunks: lists of t-indices; k's first chunk is small so the
            # transpose chain starts as early as possible.
            CHUNKS = {"k": [(0, 1, 2, 3), (4, 5, 6, 7)],
                      "q": [(0, 1, 2, 3), (4, 5, 6, 7)],
                      "v": [(0, 1, 2, 3), (4, 5, 6, 7)]}
            chunk_of = {(n, t): ci for n, chs in CHUNKS.items()
                        for ci, ts in enumerate(chs) for t in ts}
            x32 = {}   # (name, ch) -> tile
            def dma_x(name, ch):
                ts = CHUNKS[name][ch]
                t_ = x32_pool.tile([128, len(ts), C], f32,
                                   name=f"x32_{name}{ch}", tag="x32",
                                   padded_shape=[128, 4, C])
                x_r = x_d[name][:].rearrange("(t p) c -> p t c", p=128)
                nc.sync.dma_start(t_[:], x_r[:, ts[0]:ts[0] + len(ts), :])
                x32[(name, ch)] = t_

            w_sb = {}
            w16 = {}
            def dma_w(name):
                w_sb[name] = w_pool.tile([128, 4, D], f32, name=f"w_{name}",
                                         tag=f"w_{name}")
                nc.sync.dma_start(
                    w_sb[name][:],
                    w_d[name][:].rearrange("(cc p) d -> p cc d", p=128))
                w16[name] = w_pool.tile([128, 4, D], f16, name=f"w16_{name}",
                                        tag=f"w16_{name}")

            # lazy per-chunk W fp16 converts on the scalar engine
            w16_done = set()

            def cvt_w(name, dt):
                if (name, dt) in w16_done:
                    return
                w16_done.add((name, dt))
                nc.vector.tensor_copy(
                    w16[name][:, :, dt * 128:(dt + 1) * 128],
                    w_sb[name][:, :, dt * 128:(dt + 1) * 128])

            def cvt_wv_half(half):
                if ("v", half) in w16_done:
                    return
                w16_done.add(("v", half))
                nc.vector.tensor_copy(
                    w16["v"][:, :, half * 256:(half + 1) * 256],
                    w_sb["v"][:, :, half * 256:(half + 1) * 256])

            dma_x("k", 0)
            dma_x("k", 1)
            dma_w("k")
            dma_w("q")
            dma_x("q", 0)
            b_sb = {}
            for name in ("k", "q"):
                b_sb[name] = w_pool.tile([128, 4], f32, name=f"b_{name}",
                                         tag=f"b_{name}")
                nc.sync.dma_start(
                    b_sb[name][:], b_d[name][:].rearrange("(dt p) -> p dt", p=128))
            bv_sb = w_pool.tile([1, D], f32, name="bv_sb", tag="bv_sb")
            nc.sync.dma_start(
                bv_sb[:], b_d["v"][:].rearrange("(o d) -> o d", o=1))
            bv_16 = w_pool.tile([1, D], f16, name="bv_16", tag="bv_16")
            nc.scalar.copy(bv_16[:], bv_sb[:])
            dma_w("v")
            dma_x("v", 0)
            dma_x("v", 1)
            dma_x("q", 1)

            # ---------------- emission helpers ----------------
            xb = {}

            def cv(name, ch):
                """Convert x chunk to fp16 on DVE."""
                ts = CHUNKS[name][ch]
                t_ = xb_pool.tile([128, len(ts), C], f16,
                                  name=f"xb_{name}{ch}", tag="xb",
                                  padded_shape=[128, 4, C])
                nc.vector.tensor_copy(t_[:], x32[(name, ch)][:])
                xb[(name, ch)] = t_

            def T_t(name, t):
                """PE-transpose x rows t*128..t*128+128 into xT[name]."""
                ch = chunk_of[(name, t)]
                src = xb[(name, ch)]
                ti = t - CHUNKS[name][ch][0]
                pst = ps_p.tile([128, 4, 128], f16, tag="pp",
                                name=f"pst_{name}_{t}")
                for cc in range(4):
                    nc.tensor.transpose(
                        pst[:, cc, :],
                        src[:, ti, cc * 128:(cc + 1) * 128],
                        ident_f16[:])
                nc.vector.tensor_copy(
                    xT[name][:, :, t * 128:(t + 1) * 128], pst[:])

            def projqk(name, dt, qhs=(0, 1)):
                """Q/K projection d-chunk dt for the given q-halves.

                stationary: fp16 W slice, moving: fp16 xT.
                """
                tgt = QT if name == "q" else KT
                psqs = {}
                for qh in qhs:
                    psqs[qh] = ps_p.tile([128, 512], f32, tag="pp",
                                         name=f"psq_{name}{dt}_{qh}")
                for cc in range(4):
                    for qh in qhs:
                        nc.tensor.matmul(
                            psqs[qh][:],
                            w16[name][:, cc, dt * 128:(dt + 1) * 128],
                            xT[name][:, cc, qh * 512:(qh + 1) * 512],
                            start=(cc == 0), stop=(cc == 3))
                for qh in qhs:
                    nc.vector.tensor_scalar_add(
                        tgt[:, dt, qh * 512:(qh + 1) * 512], psqs[qh][:],
                        b_sb[name][:, dt:dt + 1])

            def projv(st):
                """V projection s-chunk st: stationary xT_v fp16 (FWL),
                moving W fp16; ones-column bias matmul; evac to V_aug."""
                psv = ps_p.tile([128, 512], f32, tag="pp", name=f"psv_{st}")
                for cc in range(4):
                    nc.tensor.matmul(
                        psv[:],
                        xT["v"][:, cc, st * 128:(st + 1) * 128],
                        w16["v"][:, cc, :],
                        start=(cc == 0), stop=False)
                nc.tensor.matmul(
                    psv[:], ones_16[0:1, 0:128], bv_16[0:1, :],
                    start=False, stop=True)
                nc.vector.tensor_copy(
                    V[:, st, :, 0:HD],
                    psv[:].rearrange("p (h e) -> p h e", h=HEADS))

            # ---------------- filler queue ----------------
            fillers = []

            def fill(n=1):
                for _ in range(n):
                    if fillers:
                        fillers.pop(0)()

            def drain_fillers():
                while fillers:
                    fillers.pop(0)()

            # ---------------- attention ----------------
            # Deferred tail (attV kt=7 + evacuation) per sub-block, flushed
            # inside the next sub-block after its first exp.
            pend = [None]

            def scores_exp(hp, qh, kt):
                heads = (2 * hp, 2 * hp + 1)
                pss = ps_s.tile([128, 2, 512], f32, tag="pss",
                                name=f"pss_{hp}_{qh}_{kt}")
                for i, h in enumerate(heads):
                    po = (h % 2) * HD
                    nc.tensor.matmul(
                        pss[:, i, :],
                        KT[po:po + HD, hp, kt * 128:(kt + 1) * 128],
                        QT[po:po + HD, hp, qh * 512:(qh + 1) * 512],
                        start=True, stop=True)
                attT = att_pool.tile([128, 2, 512], bf16, tag="attT",
                                     name=f"attT_{hp}_{qh}_{kt}")
                nc.scalar.activation(attT[:], pss[:], Exp, scale=0.125)
                return attT

            def attention(hp):
                heads = (2 * hp, 2 * hp + 1)
                for qh in range(2):
                    atts = []
                    atts.append(scores_exp(hp, qh, 0))
                    if pend[0] is not None:
                        pend[0]()
                        pend[0] = None
                    pso = {}
                    for h in heads:
                        pso[h] = ps_o.tile([128, 512], f32,
                                           name=f"pso{h}_{qh}", tag="pso")
                    # software pipeline: attv lags scores by two groups so
                    # the PE never waits on the exp stream.
                    for kt in range(1, 8):
                        atts.append(scores_exp(hp, qh, kt))
                        if kt >= 2:
                            for i, h in enumerate(heads):
                                nc.tensor.matmul(
                                    pso[h][:],
                                    V[:, kt - 2, h, :],
                                    atts[kt - 2][:, i, :],
                                    start=(kt == 2), stop=False)
                        fill(1)
                    for i, h in enumerate(heads):
                        nc.tensor.matmul(
                            pso[h][:], V[:, 6, h, :], atts[6][:, i, :],
                            start=False, stop=False)

                    att7 = atts[7]

                    def tail(heads=heads, pso=pso, att7=att7, qh=qh):
                        for i, h in enumerate(heads):
                            nc.tensor.matmul(
                                pso[h][:], V[:, 7, h, :], att7[:, i, :],
                                start=False, stop=True)
                        for h in heads:
                            oT = ot_pool.tile([HD + 1, 512], f16, tag="oT",
                                              name=f"oT{h}_{qh}")
                            nc.vector.tensor_copy(oT[:], pso[h][0:HD + 1, :])
                            pbt = ps_o.tile([128, 4, HD + 2], f16, tag="pso",
                                            name=f"pbt{h}_{qh}")
                            for qs in range(4):
                                nc.tensor.transpose(
                                    pbt[:, qs, 0:HD + 1],
                                    oT[:, qs * 128:(qs + 1) * 128],
                                    ident_f16[0:HD + 1, 0:HD + 1])
                            rec = ot_pool.tile([128, 4], f32, tag="rec",
                                               name=f"rec{h}_{qh}")
                            nc.vector.reciprocal(rec[:], pbt[:, :, HD])
                            for qs in range(4):
                                qt = qh * 4 + qs
                                nc.vector.tensor_scalar_mul(
                                    o_stage[:, qt, h * HD:(h + 1) * HD],
                                    pbt[:, qs, 0:HD],
                                    rec[:, qs:qs + 1])

                    pend[0] = tail

            # ---------------- prefix ----------------
            cv("k", 0)
            for t in range(4):
                T_t("k", t)
            cv("k", 1)
            cvt_w("k", 0)
            for t in range(4, 8):
                T_t("k", t)
            projqk("k", 0)
            cv("q", 0)
            cvt_w("q", 0)
            for t in range(4):
                T_t("q", t)
            projqk("q", 0, qhs=(0,))
            cvt_wv_half(0)
            cvt_wv_half(1)
            cv("v", 0)
            T_t("v", 0)
            projv(0)
            T_t("v", 1)
            projv(1)

            # ---------------- filler schedule ----------------
            def u(*fns):
                def unit():
                    for f in fns:
                        f()
                return unit

            fillers.extend([
                # during (0, qh0): keep the V st chain >= 1 ahead of attv,
                # finish xT_q and QT dt0 qh1 (needed at (0, qh1) kt0).
                u(lambda: T_t("v", 2), lambda: projv(2)),
                u(lambda: T_t("v", 3), lambda: projv(3)),
                u(lambda: cv("v", 1), lambda: T_t("v", 4), lambda: projv(4),
                  lambda: T_t("v", 5), lambda: projv(5)),
                u(lambda: T_t("v", 6), lambda: projv(6)),
                u(lambda: T_t("v", 7), lambda: projv(7)),
                u(lambda: cv("q", 1)),
                u(lambda: [T_t("q", t) for t in range(4, 8)],
                  lambda: projqk("q", 0, qhs=(1,))),
                # during (0, qh1)
                u(lambda: cvt_w("k", 1), lambda: projqk("k", 1)),
                u(lambda: cvt_w("q", 1), lambda: projqk("q", 1)),
                u(lambda: cvt_w("k", 2), lambda: projqk("k", 2)),
                u(lambda: cvt_w("q", 2), lambda: projqk("q", 2)),
                u(lambda: cvt_w("k", 3), lambda: projqk("k", 3)),
                # during (1, qh0)
                u(lambda: cvt_w("q", 3), lambda: projqk("q", 3)),
            ])

            out_r = out_d[:].rearrange("(t p) d -> p t d", p=128)

            attention(0)
            attention(1)
            # hp0 tails flushed inside attention(1)'s first sub-block
            nc.sync.dma_start(out_r[:, :, 0:128], o_stage[:, :, 0:128])
            attention(2)
            nc.sync.dma_start(out_r[:, :, 128:256], o_stage[:, :, 128:256])
            attention(3)
            nc.sync.dma_start(out_r[:, :, 256:384], o_stage[:, :, 256:384])
            # hp3 qh0 rows were evacuated by the tail flushed inside (3, qh1)
            nc.sync.dma_start(out_r[:, 0:4, 384:512], o_stage[:, 0:4, 384:512])
            pend[0]()
            pend[0] = None
            drain_fillers()
            nc.sync.dma_start(out_r[:, 4:8, 384:512], o_stage[:, 4:8, 384:512])

    nc.compile()
    return nc


_NC = None


def _get_nc():
    global _NC
    if _NC is None:
        _NC = build_nc()
    return _NC


def _make_in_maps(inputs):
    in_maps = []
    for b in range(B):
        m = {
            "q_in": np.ascontiguousarray(np.asarray(inputs["q_in"])[b].reshape(S, C)),
            "k_in": np.ascontiguousarray(np.asarray(inputs["k_in"])[b].reshape(S, C)),
            "v_in": np.ascontiguousarray(np.asarray(inputs["v_in"])[b].reshape(S, C)),
            "Wq": np.asarray(inputs["Wq"]), "bq": np.asarray(inputs["bq"]),
            "Wk": np.asarray(inputs["Wk"]), "bk": np.asarray(inputs["bk"]),
            "Wv": np.asarray(inputs["Wv"]), "bv": np.asarray(inputs["bv"]),
        }
        in_maps.append(m)
    return in_maps


def kernel(**inputs):
    nc = _get_nc()
    res = run_bass_kernel_spmd(nc, _make_in_maps(inputs), list(range(N_CORES)))
    out = np.stack([res.results[i]["out"] for i in range(B)])
    return out.reshape(B, HS, WS, D).astype(np.float32)


if __name__ == "__main__":
    rng = np.random.default_rng(0)
    ins = {
        "q_in": rng.standard_normal((B, HS, WS, C), dtype=np.float32),
        "k_in": rng.standard_normal((B, HS, WS, C), dtype=np.float32),
        "v_in": rng.standard_normal((B, HS, WS, C), dtype=np.float32),
        "Wq": (rng.standard_normal((C, D)) / np.sqrt(C)).astype(np.float32),
        "Wk": (rng.standard_normal((C, D)) / np.sqrt(C)).astype(np.float32),
        "Wv": (rng.standard_normal((C, D)) / np.sqrt(C)).astype(np.float32),
        "bq": np.zeros(D, np.float32),
        "bk": np.zeros(D, np.float32),
        "bv": np.zeros(D, np.float32),
    }
    out = kernel(**ins)
    print("out shape:", out.shape, "finite:", np.isfinite(out).all())
